# revision 7
# baseline (speedup 1.0000x reference)
"""Distributed embedding lookup (gather) for 8 Trainium2 NeuronCores.

Strategy (model-parallel, per the row-shard hint):
  - The [1M, 64] f32 table is range-sharded: core c owns rows
    [c*125000, (c+1)*125000)  (32 MB per core, nothing replicated).
  - Host routes each id to its owning core ("all-to-all" of ids done
    host-side), converts to shard-local indices, and buckets them by
    32768-row windows because the on-device gather primitive
    (InstDMAGatherAnt) takes int16 indices.
  - Window capacities are sized per call from the actual per-core counts
    (max over cores, rounded up to 256) so almost no pad slots are
    gathered; the program is compiled per capacity signature and cached.
  - On device each core pipelines:
      gpsimd (Pool):  dma_gather chunks (<=8192 rows; SWDGE ring limit)
                      table->SBUF, round-robin over NB buffers.
      sync (SP):      per-chunk idx loads DRAM->SBUF, then writes of the
                      left half of each gathered chunk SBUF->DRAM.
      scalar (Act):   writes of the right half of each chunk.
    Pool's per-row gather cost dominates; the two HWDGE engines give the
    write side 2x bandwidth so it stays hidden behind the gather.
  - Host scatters the per-core results back into the full
    [16384, 50, 64] output via a slot->original-position map.
"""

import numpy as np

import concourse.bacc as bacc
import concourse.bass as bass
import concourse.mybir as mybir
from concourse.bass_utils import run_bass_kernel_spmd

# ---- problem constants (hardcoded; kernel.py must be self-contained) ----
N_CORES = 8
VOCAB = 1_000_000
EMB = 64                      # 64 f32 = 256 B per row (dma_gather needs %256B)
ROWS_PER_CORE = VOCAB // N_CORES   # 125_000
WIN = 32768                   # int16 index window
CH_MAX = 8192                 # ids per dma_gather call (SWDGE ring limit)
NB = 8                        # SBUF destination buffers
CAP_GRAN = 128                # capacity rounding granularity

# per-core windows: (local_start, height)
WINDOWS = []
_s = 0
while _s < ROWS_PER_CORE:
    WINDOWS.append((_s, min(WIN, ROWS_PER_CORE - _s)))
    _s += WIN
# -> [(0,32768),(32768,32768),(65536,32768),(98304,26696)]
N_WIN = len(WINDOWS)


def make_chunks(caps):
    """caps -> (chunks, cap_offsets, total_slots).

    chunks: list of (window_idx, global_slot_offset, size), ordered so the
    smallest chunk runs last (shortest write tail).
    """
    cap_offsets = np.concatenate([[0], np.cumsum(caps)]).astype(np.int64)
    chunks = []
    for w, cap in enumerate(caps):
        off = int(cap_offsets[w])
        left = int(cap)
        while left > 0:
            sz = min(CH_MAX, left)
            chunks.append((w, off, sz))
            off += sz
            left -= sz
    assert all(sz % 128 == 0 for _, _, sz in chunks)
    chunks.sort(key=lambda c: -c[2])  # big first, smallest last
    if len(chunks) > 2:
        # a small chunk first shortens the pipeline fill; smallest stays
        # last for the shortest write tail
        chunks.insert(0, chunks.pop(-2))
    return chunks, cap_offsets, int(cap_offsets[-1])


def build_nc(caps):
    chunks, _, total_slots = make_chunks(caps)
    total_cols = total_slots // 16

    nc = bacc.Bacc("TRN2")
    shard = nc.dram_tensor(
        "shard", [ROWS_PER_CORE, EMB], mybir.dt.float32, kind="ExternalInput"
    )
    idxs = nc.dram_tensor(
        "idxs", [128, total_cols], mybir.dt.int16, kind="ExternalInput"
    )
    out = nc.dram_tensor(
        "out", [total_slots * EMB], mybir.dt.float32, kind="ExternalOutput"
    )

    from contextlib import ExitStack

    with ExitStack() as stack:
        block = stack.enter_context(nc.Block())
        idx_sb = stack.enter_context(
            nc.sbuf_tensor("idx_sb", [128, total_cols], mybir.dt.int16)
        )
        dsts = [
            stack.enter_context(
                nc.sbuf_tensor(f"dst{b}", [128, (CH_MAX // 128) * EMB],
                               mybir.dt.float32)
            )
            for b in range(NB)
        ]
        io_sems = [
            stack.enter_context(nc.semaphore(f"io{i}"))
            for i in range(len(chunks))
        ]
        g_sems = [stack.enter_context(nc.semaphore(f"g{b}")) for b in range(NB)]
        # separate per-writer completion sems: DMA completions across engines
        # are unordered, so summed waits on a shared sem are invalid
        o_sems = [
            [stack.enter_context(nc.semaphore(f"o{s}_{b}")) for b in range(NB)]
            for s in range(2)
        ]

        @block.gpsimd
        def _(gpsimd: bass.BassGpSimd):
            for i, (w, off, sz) in enumerate(chunks):
                b, r = i % NB, i // NB
                gpsimd.wait_ge(io_sems[i], 16)
                if i >= NB:
                    # wait until the buffer's previous contents were written
                    gpsimd.wait_ge(o_sems[0][b], 16 * r)
                    gpsimd.wait_ge(o_sems[1][b], 16 * r)
                wstart, wh = WINDOWS[w]
                dst_ap = dsts[b][:, : (sz // 128) * EMB].rearrange(
                    "p (a e) -> p a e", e=EMB
                )
                gpsimd.dma_gather(
                    dst_ap,
                    shard[wstart : wstart + wh, :],
                    idx_sb[:, off // 16 : (off + sz) // 16],
                    sz,
                    sz,
                    EMB,
                    single_packet=False,  # single-packet caps out ~1-2K idxs
                ).then_inc(g_sems[b], 16)

        def writer(side):
            """side 0 -> left half columns (SP), side 1 -> right half (Act)."""

            def go(eng: bass.BassEngine):
                # both writers stream idx chunks, alternating, in gather order
                for i, (w, off, sz) in enumerate(chunks):
                    if i % 2 == side:
                        eng.dma_start(
                            idx_sb[:, off // 16 : (off + sz) // 16],
                            idxs[:, off // 16 : (off + sz) // 16],
                        ).then_inc(io_sems[i], 16)
                uses = [0] * NB
                for i, (w, off, sz) in enumerate(chunks):
                    b, r = i % NB, i // NB
                    eng.wait_ge(g_sems[b], 16 * (r + 1))
                    half = (sz // 128) * EMB // 2
                    sl = slice(0, half) if side == 0 else slice(half, 2 * half)
                    dst = out[off * EMB : (off + sz) * EMB].rearrange(
                        "(p f) -> p f", p=128
                    )[:, sl]
                    eng.dma_start(dst, dsts[b][:, sl]).then_inc(
                        o_sems[side][b], 16
                    )
                    uses[b] += 1
                if side == 0:
                    for s in range(2):
                        for b in range(NB):
                            eng.wait_ge(o_sems[s][b], 16 * uses[b])
            return go

        block.sync(writer(0))
        block.scalar(writer(1))

    nc.compile()
    return nc


_NC_CACHE = {}
LAST_RESULTS = None  # BassKernelResults of the most recent run (for test.py)
RUN_WALL_S = -1.0    # wall time of the device dispatch+exec (for test.py)


def _get_nc(caps):
    key = tuple(int(c) for c in caps)
    if key not in _NC_CACHE:
        _NC_CACHE[key] = build_nc(key)
    return _NC_CACHE[key]


def _route(flat_ids):
    """Route ids to cores/windows/slots.

    Returns (caps, idx_tensors, slot_maps) where
      caps:        [N_WIN] per-window slot capacity (max over cores, padded)
      idx_tensors: list of [128, total_cols] int16 per core
      slot_maps:   list of [total_slots] int64 per core (orig flat pos, -1 pad)
    """
    owner = flat_ids // ROWS_PER_CORE
    local = flat_ids - owner * ROWS_PER_CORE
    win = local // WIN
    key = owner * N_WIN + win
    order = np.argsort(key, kind="stable")
    counts = np.bincount(key, minlength=N_CORES * N_WIN).reshape(
        N_CORES, N_WIN
    )
    caps = (
        (counts.max(axis=0) + CAP_GRAN - 1) // CAP_GRAN * CAP_GRAN
    ).astype(np.int64)
    caps = np.maximum(caps, 128)

    chunks, cap_offsets, total_slots = make_chunks(caps)
    total_cols = total_slots // 16

    starts = np.concatenate([[0], np.cumsum(counts.reshape(-1))])
    idx_tensors, slot_maps = [], []
    for c in range(N_CORES):
        slot_ids = np.zeros(total_slots, np.int16)
        slot_pos = np.full(total_slots, -1, np.int64)
        for wi in range(N_WIN):
            k = c * N_WIN + wi
            seg_pos = order[starts[k] : starts[k + 1]]
            n = len(seg_pos)
            base = int(cap_offsets[wi])
            slot_ids[base : base + n] = (
                local[seg_pos] - WINDOWS[wi][0]
            ).astype(np.int16)
            slot_pos[base : base + n] = seg_pos

        # per-chunk 16-partition wrap: slot j of a chunk -> [j%16, j//16]
        cols = np.empty((16, total_cols), np.int16)
        for _, off, sz in chunks:
            cols[:, off // 16 : (off + sz) // 16] = (
                slot_ids[off : off + sz].reshape(sz // 16, 16).T
            )
        idx_tensors.append(np.tile(cols, (8, 1)))  # replicate to 128 parts
        slot_maps.append(slot_pos)

    return caps, idx_tensors, slot_maps


def make_in_maps(ids_np, table_np):
    """Host-side routing: full inputs -> (caps, per-core in_maps, slot maps)."""
    flat = ids_np.reshape(-1).astype(np.int64)
    caps, idx_tensors, slot_maps = _route(flat)
    in_maps = [
        {
            "shard": np.ascontiguousarray(
                table_np[c * ROWS_PER_CORE : (c + 1) * ROWS_PER_CORE]
            ),
            "idxs": idx_tensors[c],
        }
        for c in range(N_CORES)
    ]
    return caps, in_maps, slot_maps


def kernel(ids, table):
    ids_np = np.asarray(ids)
    table_np = np.asarray(table, dtype=np.float32)
    n = int(np.prod(ids_np.shape))

    caps, in_maps, slot_maps = make_in_maps(ids_np, table_np)
    chunks, _, total_slots = make_chunks(caps)

    nc = _get_nc(caps)
    import time as _time

    _t0 = _time.time()
    res = run_bass_kernel_spmd(nc, in_maps, core_ids=list(range(N_CORES)))
    global LAST_RESULTS, RUN_WALL_S
    RUN_WALL_S = _time.time() - _t0
    LAST_RESULTS = res

    out_flat = np.empty((n, EMB), np.float32)
    for c in range(N_CORES):
        o = np.asarray(res.results[c]["out"]).reshape(-1)
        rows = np.empty((total_slots, EMB), np.float32)
        for _, off, sz in chunks:
            blk = o[off * EMB : (off + sz) * EMB].reshape(128, sz // 128, EMB)
            rows[off : off + sz] = blk.transpose(1, 0, 2).reshape(sz, EMB)
        valid = slot_maps[c] >= 0
        out_flat[slot_maps[c][valid]] = rows[valid]

    return out_flat.reshape(*ids_np.shape, EMB)


# revision 17
# speedup vs baseline: 1.0166x; 1.0166x over previous
"""Distributed embedding lookup (gather) for 8 Trainium2 NeuronCores.

Strategy (model-parallel, per the row-shard hint):
  - The [1M, 64] f32 table is range-sharded: core c owns rows
    [c*125000, (c+1)*125000)  (32 MB per core, nothing replicated).
  - Host routes each id to its owning core ("all-to-all" of ids done
    host-side), converts to shard-local indices, and buckets them by
    32768-row windows because the on-device gather primitive
    (InstDMAGatherAnt) takes int16 indices.
  - Window capacities are sized per call from the actual per-core counts
    (max over cores, rounded up to 128) so almost no pad slots are
    gathered; the program is compiled per capacity signature and cached.
  - On device each core pipelines three DMA-capable engines:
      gpsimd (Pool):  dma_gather chunks (<=8192 rows; SWDGE ring limit)
                      table->SBUF, round-robin over NB buffers, then the
                      final tail chunk's write itself.
      sync (SP) and scalar (Act): idx loads DRAM->SBUF in geometric
                      groups (every DMA costs >=500ns of engine time, so
                      few big loads beat many small ones), then writes of
                      half of each gathered chunk SBUF->DRAM; the last
                      three small chunks are whole-chunk writes on SP,
                      Act and Pool so all engines finish together.
    The gather's per-row cost on Pool dominates and runs gap-free; the
    chunk list starts with a small ramp so the pipeline fills while the
    first idx groups load.
  - Host scatters the per-core results back into the full
    [16384, 50, 64] output via a slot->original-position map.
"""

import numpy as np

import concourse.bacc as bacc
import concourse.bass as bass
import concourse.mybir as mybir
from concourse.bass_utils import run_bass_kernel_spmd

# ---- problem constants (hardcoded; kernel.py must be self-contained) ----
N_CORES = 8
VOCAB = 1_000_000
EMB = 64                      # 64 f32 = 256 B per row (dma_gather needs %256B)
ROWS_PER_CORE = VOCAB // N_CORES   # 125_000
WIN = 32768                   # int16 index window
CH_MAX = 8192                 # ids per dma_gather call (SWDGE ring limit)
NB = 8                        # SBUF destination buffers
CAP_GRAN = 128                # capacity rounding granularity

# per-core windows: (local_start, height)
WINDOWS = []
_s = 0
while _s < ROWS_PER_CORE:
    WINDOWS.append((_s, min(WIN, ROWS_PER_CORE - _s)))
    _s += WIN
# -> [(0,32768),(32768,32768),(65536,32768),(98304,26696)]
N_WIN = len(WINDOWS)


def _split_sizes(total, head=(), tail=()):
    """Split `total` into sizes: optional head ramp, <=CH_MAX bodies, tail."""
    head = [h for h in head]
    tail = [t for t in tail]
    if total < sum(head) + sum(tail) + 128:
        head, tail = [], []  # degenerate small window: plain split
    body = total - sum(head) - sum(tail)
    sizes = list(head)
    while body > 0:
        sz = min(CH_MAX, body)
        if body - sz and body - sz < 128:
            sz -= 128
        sizes.append(sz)
        body -= sz
    sizes.extend(tail)
    assert sum(sizes) == total and all(s % 128 == 0 and s > 0 for s in sizes)
    return sizes


def make_chunks(caps):
    """caps -> (chunks, cap_offsets, total_slots).

    chunks: list of (window_idx, global_slot_offset, size) in processing
    order == global slot order.  The first window starts with a ramp of
    small gathers (so the pipeline fills while idx loads stream in) and the
    last window ends with two small chunks (so the trailing write after the
    final gather is short).  Gather instruction count is free on Pool; each
    DMA costs >=500ns on the issuing engine, which sets the ramp scale.
    """
    cap_offsets = np.concatenate([[0], np.cumsum(caps)]).astype(np.int64)
    chunks = []
    last = len(caps) - 1
    for w, cap in enumerate(caps):
        head = (1024, 2048, 4096) if w == 0 else ()
        tail = (2688, 2688, 2304) if w == last else ()
        off = int(cap_offsets[w])
        for sz in _split_sizes(int(cap), head, tail):
            chunks.append((w, off, sz))
            off += sz
    return chunks, cap_offsets, int(cap_offsets[-1])


def make_writer_plan(chunks):
    """Writer assignment per chunk: list of (writer, frac_lo, frac_hi).

    writer 0 = SP, 1 = Act, 2 = Pool.  Steady-state chunks are half-split
    between SP and Act.  The last three chunks are whole-chunk writes on
    SP, Act, and Pool so the trailing write after the final gather is done
    by the otherwise-idle Pool engine while SP/Act drain their backlog.
    """
    n = len(chunks)
    plan = []
    for i in range(n):
        sz = chunks[i][2]
        if n >= 6 and sz <= 4096 and i == n - 3:
            plan.append([(0, 0.0, 1.0)])
        elif n >= 6 and sz <= 4096 and i == n - 2:
            plan.append([(1, 0.0, 1.0)])
        elif n >= 6 and sz <= 4096 and i == n - 1:
            plan.append([(2, 0.0, 1.0)])
        else:
            plan.append([(0, 0.0, 0.5), (1, 0.5, 1.0)])
    return plan


def make_load_groups(total_slots):
    """Geometric idx-load column groups [start_col, end_col) in slot order."""
    bounds = []
    acc = 0
    step = 2048
    while acc < total_slots:
        acc = min(acc + step, total_slots)
        if total_slots - acc < 2048:
            acc = total_slots
        bounds.append(acc)
        step = min(step * 2, 40960)
    groups = []
    prev = 0
    for b in bounds:
        groups.append((prev // 16, b // 16))
        prev = b
    return groups


def build_nc(caps):
    chunks, _, total_slots = make_chunks(caps)
    total_cols = total_slots // 16

    nc = bacc.Bacc("TRN2")
    shard = nc.dram_tensor(
        "shard", [ROWS_PER_CORE, EMB], mybir.dt.float32, kind="ExternalInput"
    )
    idxs = nc.dram_tensor(
        "idxs", [128, total_cols], mybir.dt.int16, kind="ExternalInput"
    )
    out = nc.dram_tensor(
        "out", [total_slots * EMB], mybir.dt.float32, kind="ExternalOutput"
    )

    from contextlib import ExitStack

    with ExitStack() as stack:
        block = stack.enter_context(nc.Block())
        idx_sb = stack.enter_context(
            nc.sbuf_tensor("idx_sb", [128, total_cols], mybir.dt.int16)
        )
        dsts = [
            stack.enter_context(
                nc.sbuf_tensor(f"dst{b}", [128, (CH_MAX // 128) * EMB],
                               mybir.dt.float32)
            )
            for b in range(NB)
        ]
        groups = make_load_groups(total_slots)
        plan = make_writer_plan(chunks)
        io_sems = [
            stack.enter_context(nc.semaphore(f"io{i}"))
            for i in range(len(groups))
        ]
        g_sems = [stack.enter_context(nc.semaphore(f"g{b}")) for b in range(NB)]
        # separate per-writer completion sems: DMA completions across engines
        # are unordered, so summed waits on a shared sem are invalid
        o_sems = [
            [stack.enter_context(nc.semaphore(f"o{s}_{b}")) for b in range(NB)]
            for s in range(3)
        ]

        total_uses = [[0] * NB for _ in range(3)]
        for j, _c in enumerate(chunks):
            for (s, _, _) in plan[j]:
                total_uses[s][j % NB] += 1

        def piece_slice(sz, lo, hi):
            free = (sz // 128) * EMB
            return slice(int(free * lo), int(free * hi))

        @block.gpsimd
        def _(gpsimd: bass.BassGpSimd):
            waited = 0
            cnt = [[0] * NB for _ in range(3)]  # writes issued per (side, buf)
            for i, (w, off, sz) in enumerate(chunks):
                b, r = i % NB, i // NB
                need_col = (off + sz) // 16
                while waited < len(groups) and groups[waited][0] < need_col:
                    gpsimd.wait_ge(io_sems[waited], 16)
                    waited += 1
                if i >= NB:
                    # wait until the buffer's previous contents were written
                    for (s, _, _) in plan[i - NB]:
                        gpsimd.wait_ge(o_sems[s][b], 16 * cnt[s][b])
                for (s, _, _) in plan[i]:
                    cnt[s][b] += 1
                wstart, wh = WINDOWS[w]
                dst_ap = dsts[b][:, : (sz // 128) * EMB].rearrange(
                    "p (a e) -> p a e", e=EMB
                )
                gpsimd.dma_gather(
                    dst_ap,
                    shard[wstart : wstart + wh, :],
                    idx_sb[:, off // 16 : (off + sz) // 16],
                    sz,
                    sz,
                    EMB,
                    single_packet=False,  # single-packet caps out ~1-2K idxs
                ).then_inc(g_sems[b], 16)
            # Pool takes the writes assigned to it (the final tail chunk)
            done = [0] * NB
            for i, (w, off, sz) in enumerate(chunks):
                b, r = i % NB, i // NB
                for (s, lo, hi) in plan[i]:
                    if s != 2:
                        continue
                    gpsimd.wait_ge(g_sems[b], 16 * (r + 1))
                    sl = piece_slice(sz, lo, hi)
                    dst = out[off * EMB : (off + sz) * EMB].rearrange(
                        "(p f) -> p f", p=128
                    )[:, sl]
                    gpsimd.dma_start(dst, dsts[b][:, sl]).then_inc(
                        o_sems[2][b], 16
                    )

        def writer(side):
            def go(eng: bass.BassEngine):
                # both writers stream idx load groups, alternating, in order
                for gi, (c0, c1) in enumerate(groups):
                    if gi % 2 == side:
                        eng.dma_start(
                            idx_sb[:, c0:c1], idxs[:, c0:c1]
                        ).then_inc(io_sems[gi], 16)
                for i, (w, off, sz) in enumerate(chunks):
                    b, r = i % NB, i // NB
                    for (s, lo, hi) in plan[i]:
                        if s != side:
                            continue
                        eng.wait_ge(g_sems[b], 16 * (r + 1))
                        sl = piece_slice(sz, lo, hi)
                        dst = out[off * EMB : (off + sz) * EMB].rearrange(
                            "(p f) -> p f", p=128
                        )[:, sl]
                        eng.dma_start(dst, dsts[b][:, sl]).then_inc(
                            o_sems[side][b], 16
                        )
                if side == 0:
                    for s in range(3):
                        for b in range(NB):
                            if total_uses[s][b] > 0:
                                eng.wait_ge(o_sems[s][b], 16 * total_uses[s][b])
            return go

        block.sync(writer(0))
        block.scalar(writer(1))

    nc.compile()
    return nc


_NC_CACHE = {}
LAST_RESULTS = None  # BassKernelResults of the most recent run (for test.py)
RUN_WALL_S = -1.0    # wall time of the device dispatch+exec (for test.py)


def _get_nc(caps):
    key = tuple(int(c) for c in caps)
    if key not in _NC_CACHE:
        _NC_CACHE[key] = build_nc(key)
    return _NC_CACHE[key]


def _route(flat_ids):
    """Route ids to cores/windows/slots.

    Returns (caps, idx_tensors, slot_maps) where
      caps:        [N_WIN] per-window slot capacity (max over cores, padded)
      idx_tensors: list of [128, total_cols] int16 per core
      slot_maps:   list of [total_slots] int64 per core (orig flat pos, -1 pad)
    """
    owner = flat_ids // ROWS_PER_CORE
    local = flat_ids - owner * ROWS_PER_CORE
    win = local // WIN
    key = owner * N_WIN + win
    order = np.argsort(key, kind="stable")
    counts = np.bincount(key, minlength=N_CORES * N_WIN).reshape(
        N_CORES, N_WIN
    )
    caps = (
        (counts.max(axis=0) + CAP_GRAN - 1) // CAP_GRAN * CAP_GRAN
    ).astype(np.int64)
    caps = np.maximum(caps, 128)

    chunks, cap_offsets, total_slots = make_chunks(caps)
    total_cols = total_slots // 16

    starts = np.concatenate([[0], np.cumsum(counts.reshape(-1))])
    idx_tensors, slot_maps = [], []
    for c in range(N_CORES):
        slot_ids = np.zeros(total_slots, np.int16)
        slot_pos = np.full(total_slots, -1, np.int64)
        for wi in range(N_WIN):
            k = c * N_WIN + wi
            seg_pos = order[starts[k] : starts[k + 1]]
            n = len(seg_pos)
            base = int(cap_offsets[wi])
            slot_ids[base : base + n] = (
                local[seg_pos] - WINDOWS[wi][0]
            ).astype(np.int16)
            slot_pos[base : base + n] = seg_pos

        # per-chunk 16-partition wrap: slot j of a chunk -> [j%16, j//16]
        cols = np.empty((16, total_cols), np.int16)
        for _, off, sz in chunks:
            cols[:, off // 16 : (off + sz) // 16] = (
                slot_ids[off : off + sz].reshape(sz // 16, 16).T
            )
        idx_tensors.append(np.tile(cols, (8, 1)))  # replicate to 128 parts
        slot_maps.append(slot_pos)

    return caps, idx_tensors, slot_maps


def make_in_maps(ids_np, table_np):
    """Host-side routing: full inputs -> (caps, per-core in_maps, slot maps)."""
    flat = ids_np.reshape(-1).astype(np.int64)
    caps, idx_tensors, slot_maps = _route(flat)
    in_maps = [
        {
            "shard": np.ascontiguousarray(
                table_np[c * ROWS_PER_CORE : (c + 1) * ROWS_PER_CORE]
            ),
            "idxs": idx_tensors[c],
        }
        for c in range(N_CORES)
    ]
    return caps, in_maps, slot_maps


def kernel(ids, table):
    ids_np = np.asarray(ids)
    table_np = np.asarray(table, dtype=np.float32)
    n = int(np.prod(ids_np.shape))

    caps, in_maps, slot_maps = make_in_maps(ids_np, table_np)
    chunks, _, total_slots = make_chunks(caps)

    nc = _get_nc(caps)
    import time as _time

    _t0 = _time.time()
    res = run_bass_kernel_spmd(nc, in_maps, core_ids=list(range(N_CORES)))
    global LAST_RESULTS, RUN_WALL_S
    RUN_WALL_S = _time.time() - _t0
    LAST_RESULTS = res

    out_flat = np.empty((n, EMB), np.float32)
    for c in range(N_CORES):
        o = np.asarray(res.results[c]["out"]).reshape(-1)
        rows = np.empty((total_slots, EMB), np.float32)
        for _, off, sz in chunks:
            blk = o[off * EMB : (off + sz) * EMB].reshape(128, sz // 128, EMB)
            rows[off : off + sz] = blk.transpose(1, 0, 2).reshape(sz, EMB)
        valid = slot_maps[c] >= 0
        out_flat[slot_maps[c][valid]] = rows[valid]

    return out_flat.reshape(*ids_np.shape, EMB)
